# revision 8
# baseline (speedup 1.0000x reference)
"""MHA kernel for trn2, 8 NeuronCores, head-sharded (2 heads/core).

Per core c (heads 2c, 2c+1):
  qT/kT/vT = (w shard).T @ x.T  -> [128, T] (rows 0:64 head a, 64:128 head b)
  vT is PE-transposed per 128-tok block into Vp = [V | ones]  [tok, 65]
  per q-tile (512) x key-block (128):
     S^T = kT_blk.T @ qT   (row-tiled head pair, K=64 per head, [128,1024] psum)
     eS  = exp(S^T + BIAS):  kb < 20: ACT exp -> bf16
                             kb >= 20: DVE Schraudolph (affine -> int16 bits
                             viewed as bf16) -- offloads ACT, drains it before
                             the q-tile boundary so the yu casts start at once
     y'[65,512] += Vp[kb].T @ eS   (row 64 = Z = sum exp)
  y freed fast via unnormalized bf16 casts (ACT); Z row spread to [64,16] by
  sbuf DMA -> parallel reciprocal -> DMA back -> gpsimd partition_broadcast ->
  DVE multiply => yns [128,512] bf16 (normalized, both heads)
  fused out-proj (K=128 = both heads): outT[:, qtile] = wo.T @ yns
Host: sums the 8 cores' outT partials.
"""

import numpy as np
import ml_dtypes

import concourse.bacc as bacc
import concourse.mybir as mybir
from concourse.tile import TileContext
from concourse.bass_utils import run_bass_kernel_spmd

BF16 = ml_dtypes.bfloat16
F32 = mybir.dt.float32
BF = mybir.dt.bfloat16
I16 = mybir.dt.int16
EXP = mybir.ActivationFunctionType.Exp
MULT = mybir.AluOpType.mult
ADD = mybir.AluOpType.add

B, T, C = 1, 4096, 1024
H, D = 16, 64
NCORES = 8
P = 128
CB = C // P          # 8 contraction blocks
KB = T // P          # 32 key blocks
QTS = T // 512       # 8 q tiles

BIAS = -3.75
A16C = 128.0 / np.log(2.0)
B16 = 16248.65 + A16C * BIAS

N_D16 = 12           # kb >= KB - N_D16 use the DVE Schraudolph exp

_cached = None


def build_bass():
    global _cached
    if _cached is not None:
        return _cached

    nc = bacc.Bacc("TRN2", target_bir_lowering=False, name="mha_v3")

    xT = nc.dram_tensor("xT", (C, T), BF, kind="ExternalInput")
    wq = nc.dram_tensor("wq", (C, P), BF, kind="ExternalInput")
    wk = nc.dram_tensor("wk", (C, P), BF, kind="ExternalInput")
    wv = nc.dram_tensor("wv", (C, P), BF, kind="ExternalInput")
    wo = nc.dram_tensor("wo", (P, C), BF, kind="ExternalInput")
    ident = nc.dram_tensor("ident", (P, P), BF, kind="ExternalInput")
    outT = nc.dram_tensor("outT", (C, T), BF, kind="ExternalOutput")

    with TileContext(nc) as tc:
        with (
            tc.tile_pool(name="const", bufs=1) as const,
            tc.tile_pool(name="work", bufs=3) as work,
            tc.tile_pool(name="psS", bufs=2, space="PSUM") as psS,
            tc.tile_pool(name="psY", bufs=1, space="PSUM") as psY,
            tc.tile_pool(name="psO", bufs=2, space="PSUM") as psO,
        ):
            # ---- load inputs (weights on gpsimd queue, x on sync queue) ----
            wqs = const.tile([P, CB, P], BF)
            nc.gpsimd.dma_start(wqs[:], wq[:, :].rearrange("(cb p) f -> p cb f", p=P))
            wks = const.tile([P, CB, P], BF)
            nc.gpsimd.dma_start(wks[:], wk[:, :].rearrange("(cb p) f -> p cb f", p=P))
            wvs = const.tile([P, CB, P], BF)
            nc.gpsimd.dma_start(wvs[:], wv[:, :].rearrange("(cb p) f -> p cb f", p=P))
            wos = const.tile([P, C], BF)
            nc.gpsimd.dma_start(wos[:], wo[:, :])
            idt = const.tile([P, P], BF)
            nc.gpsimd.dma_start(idt[:], ident[:, :])
            xTs = const.tile([P, CB, T], BF)
            xT_r = xT[:, :].rearrange("(cb p) t -> p cb t", p=P)
            for tt in range(QTS):
                nc.sync.dma_start(xTs[:, :, tt * 512:(tt + 1) * 512],
                                  xT_r[:, :, tt * 512:(tt + 1) * 512])

            bias_t = const.tile([P, 1], F32)
            nc.vector.memset(bias_t[:], BIAS)

            # ---- warm up the PE clock (HAM) during the input DMA wait ----
            warm = const.tile([P, 512], BF)
            nc.vector.memset(warm[:], 0.0)
            for _ in range(30):
                pw = psO.tile([P, 512], F32, tag="po", name="pw")
                nc.tensor.matmul(pw[:, :], warm[:, 0:P], warm[:, :],
                                 start=True, stop=True)

            qTs = const.tile([P, T], BF)
            kTs = const.tile([P, T], BF)
            eS_store = {}

            def emit_s_exp(qt, kb):
                q0 = qt * 512
                k0 = kb * P
                s = psS.tile([P, 1024], F32, tag="s", name="s")
                nc.tensor.matmul(s[:, 0:512], kTs[0:64, k0:k0 + P],
                                 qTs[0:64, q0:q0 + 512], start=True, stop=True)
                nc.tensor.matmul(s[:, 512:1024], kTs[64:128, k0:k0 + P],
                                 qTs[64:128, q0:q0 + 512], start=True, stop=True)
                eS = work.tile([P, 1024], BF, tag="es", bufs=17)
                if kb >= KB - N_D16:
                    nc.vector.tensor_scalar(eS[:].bitcast(I16), s[:], A16C, B16,
                                            op0=MULT, op1=ADD)
                else:
                    nc.scalar.activation(eS[:], s[:], EXP, bias=bias_t[:])
                eS_store[(qt, kb)] = eS

            LAG = 14

            Vp0 = const.tile([P, KB, 65], BF)
            Vp1 = const.tile([P, KB, 65], BF)
            nc.vector.memset(Vp0[:, :, 64:65], 1.0)
            nc.vector.memset(Vp1[:, :, 64:65], 1.0)

            # ---- projection phase: kT, qT, vT per tok-tile, chasing the DMA.
            # vT is transposed into Vp via the PE (identity trick).
            for tt in range(QTS):
                pq = psS.tile([P, 1024], F32, tag="s")
                for cb in range(CB):
                    nc.tensor.matmul(
                        pq[:, 0:512], wks[:, cb, :], xTs[:, cb, tt * 512:(tt + 1) * 512],
                        start=(cb == 0), stop=(cb == CB - 1))
                for cb in range(CB):
                    nc.tensor.matmul(
                        pq[:, 512:1024], wqs[:, cb, :], xTs[:, cb, tt * 512:(tt + 1) * 512],
                        start=(cb == 0), stop=(cb == CB - 1))
                nc.scalar.copy(kTs[:, tt * 512:(tt + 1) * 512], pq[:, 0:512])
                nc.vector.tensor_copy(qTs[:, tt * 512:(tt + 1) * 512], pq[:, 512:1024])

                pvt = psO.tile([P, 512], F32, tag="po", name="pvt")
                for cb in range(CB):
                    nc.tensor.matmul(
                        pvt[:, :], wvs[:, cb, :], xTs[:, cb, tt * 512:(tt + 1) * 512],
                        start=(cb == 0), stop=(cb == CB - 1))
                vts = work.tile([P, 512], BF, tag="vts", bufs=2)
                nc.vector.tensor_copy(vts[:], pvt[:])
                for j in range(4):
                    tb = tt * 4 + j
                    pvb = psO.tile([P, P], BF, tag="po", name="pvb")
                    nc.tensor.transpose(pvb[:, :], vts[:, j * P:(j + 1) * P], idt[:])
                    nc.scalar.copy(Vp0[:, tb, 0:64], pvb[:, 0:64])
                    nc.vector.tensor_copy(Vp1[:, tb, 0:64], pvb[:, 64:128])
                # pre-emit S^T+exp pairs of q-tile 0 so ACT starts early
                if tt < 7:
                    for kb_pre in range(tt * 2, tt * 2 + 2):
                        emit_s_exp(0, kb_pre)

            # ---- fused out-projection: both heads in one K=128 matmul ----
            def emit_outproj(dep, fb, flush=False):
                yns, q0 = dep
                po = psO.tile([P, 512], F32, tag="po", name="po")
                nc.tensor.matmul(po[:, :], wos[:, fb * P:(fb + 1) * P], yns[:, :],
                                 start=True, stop=True)
                oc = work.tile([P, 512], BF, tag="oc", bufs=8)
                if flush:
                    if fb % 2 == 1:
                        nc.scalar.copy(oc[:], po[:])
                    else:
                        nc.vector.tensor_copy(oc[:], po[:])
                else:
                    nc.vector.tensor_copy(oc[:], po[:])
                if fb % 2 == 0:
                    nc.gpsimd.dma_start(outT[fb * P:(fb + 1) * P, q0:q0 + 512], oc[:])
                else:
                    nc.sync.dma_start(outT[fb * P:(fb + 1) * P, q0:q0 + 512], oc[:])

            OP_SLOTS = {12: 0, 14: 1, 16: 2, 18: 3, 20: 4, 22: 5, 24: 6, 26: 7}

            pending = None
            for qt in range(QTS):
                q0 = qt * 512
                y0 = psY.tile([65, 512], F32, tag="y0")
                y1 = psY.tile([65, 512], F32, tag="y1")
                for kb in range(KB):
                    tgt = qt * KB + kb + LAG
                    if tgt < QTS * KB:
                        emit_s_exp(tgt // KB, tgt % KB)
                    eS = eS_store.pop((qt, kb))
                    nc.tensor.matmul(y0[:, :], Vp0[:, kb, :], eS[:, 0:512],
                                     start=(kb == 0), stop=(kb == KB - 1))
                    nc.tensor.matmul(y1[:, :], Vp1[:, kb, :], eS[:, 512:1024],
                                     start=(kb == 0), stop=(kb == KB - 1))
                    if pending is not None and kb in OP_SLOTS:
                        emit_outproj(pending, OP_SLOTS[kb])

                # ---- free y psum fast: unnormalized casts (ACT is drained
                # at the qt boundary because the last 12 kbs ran on DVE) ----
                yu0 = work.tile([64, 512], BF, tag="yu0", bufs=2)
                nc.scalar.copy(yu0[:, :], y0[0:64, :])
                yu1 = work.tile([64, 512], BF, tag="yu1", bufs=2)
                nc.scalar.copy(yu1[:, :], y1[0:64, :])
                zf = work.tile([65, 1024], F32, tag="zf", bufs=2)
                nc.scalar.copy(zf[64:65, 0:512], y0[64:65, :])
                nc.scalar.copy(zf[64:65, 512:1024], y1[64:65, :])

                # ---- Z -> [64,16] spread via DMA so reciprocal is parallel ----
                zsp = work.tile([64, 16], F32, tag="zsp", bufs=2)
                nc.sync.dma_start(
                    zsp[:, 0:8],
                    zf[64:65, 0:512].rearrange("o (p f) -> o p f", p=64))
                nc.sync.dma_start(
                    zsp[:, 8:16],
                    zf[64:65, 512:1024].rearrange("o (p f) -> o p f", p=64))
                zrc = work.tile([64, 16], F32, tag="zrc", bufs=2)
                nc.vector.reciprocal(zrc[:], zsp[:])
                zrw = work.tile([1, 1024], F32, tag="zrw", bufs=2)
                nc.gpsimd.dma_start(
                    zrw[0:1, 0:512].rearrange("o (p f) -> o p f", p=64),
                    zrc[:, 0:8])
                nc.gpsimd.dma_start(
                    zrw[0:1, 512:1024].rearrange("o (p f) -> o p f", p=64),
                    zrc[:, 8:16])
                zb = work.tile([64, 1024], F32, tag="zb", bufs=2)
                nc.gpsimd.partition_broadcast(zb[:, 0:512], zrw[0:1, 0:512])
                nc.gpsimd.partition_broadcast(zb[:, 512:1024], zrw[0:1, 512:1024])
                yns = work.tile([P, 512], BF, tag="yns", bufs=2)
                nc.vector.tensor_tensor(yns[0:64, :], yu0[:, :], zb[:, 0:512],
                                        op=MULT)
                y1t = work.tile([64, 512], BF, tag="y1t", bufs=2)
                nc.vector.tensor_tensor(y1t[:], yu1[:, :], zb[:, 512:1024],
                                        op=MULT)
                nc.gpsimd.dma_start(yns[64:128, :], y1t[:])
                pending = (yns, q0)

            for fb in range(CB):
                emit_outproj(pending, fb, flush=True)

    nc.compile()
    _cached = nc
    return nc


def make_in_maps(x, w_qkv, w_out):
    """x [1,T,C] f32, w_qkv [C, 3C] f32, w_out [C, C] f32 -> per-core input dicts."""
    x = np.asarray(x, dtype=np.float32)
    w_qkv = np.asarray(w_qkv, dtype=np.float32)
    w_out = np.asarray(w_out, dtype=np.float32)
    scale = 1.0 / np.sqrt(np.float32(D))
    xT = np.ascontiguousarray(x.reshape(T, C).T).astype(BF16)  # [C, T]
    ident = np.eye(P, dtype=np.float32).astype(BF16)
    in_maps = []
    for c in range(NCORES):
        cols = slice(P * c, P * (c + 1))
        wqc = np.ascontiguousarray(w_qkv[:, 0:C][:, cols] * scale).astype(BF16)
        wkc = np.ascontiguousarray(w_qkv[:, C:2 * C][:, cols]).astype(BF16)
        wvc = np.ascontiguousarray(w_qkv[:, 2 * C:3 * C][:, cols]).astype(BF16)
        woc = np.ascontiguousarray(w_out[P * c:P * (c + 1), :]).astype(BF16)
        in_maps.append({"xT": xT, "wq": wqc, "wk": wkc, "wv": wvc, "wo": woc,
                        "ident": ident})
    return in_maps


def run(x, w_qkv, w_out, trace=False):
    nc = build_bass()
    in_maps = make_in_maps(x, w_qkv, w_out)
    res = run_bass_kernel_spmd(nc, in_maps, core_ids=list(range(NCORES)), trace=trace)
    acc = np.zeros((C, T), dtype=np.float32)
    for r in res.results:
        acc += r["outT"].astype(np.float32)
    out = np.ascontiguousarray(acc.T).reshape(B, T, C)
    return out, res


def kernel(x, w_qkv, w_out):
    out, _ = run(x, w_qkv, w_out, trace=False)
    return out


# revision 9
# speedup vs baseline: 1.2185x; 1.2185x over previous
"""MHA kernel for trn2, 8 NeuronCores, head-sharded (2 heads/core).

Per core c (heads 2c, 2c+1):
  qT/kT = (w_{q,k} shard).T @ x.T  -> [128, T] bf16 (rows 0:64 head a, 64:128 head b)
  v     = x @ w_v shard            -> [T, 128]
  per q-tile (512) x key-block (128):
     S^T = kT_blk.T @ qT   (row-tiled head pair, K=64 per head, [128,1024] psum)
     eS  = exp(S^T): most kb on ACT (table exp); ~1/3 of kb on DVE via a
           Schraudolph-style affine (s -> int16 bits viewed as bf16), which
           offloads the ACT bottleneck at ~1.8% relative error on those blocks
     y'[65,512] += Vp[kb].T @ eS   (Vp = [V | ones]; row 64 = Z = sum exp)
  y psum freed fast via unnormalized bf16 casts (ACT); Z row spread to
  [64,16] by sbuf DMA -> parallel reciprocal -> DMA back -> gpsimd
  partition_broadcast -> DVE multiply => yns [128,512] bf16 (normalized)
  fused out-proj (K=128 = both heads at once): outT[:, qtile] = wo.T @ yns
Host: sums the 8 cores' outT partials (no normalization needed host-side).
"""

import numpy as np
import ml_dtypes

import concourse.bacc as bacc
import concourse.mybir as mybir
from concourse.tile import TileContext
from concourse.bass_utils import run_bass_kernel_spmd

BF16 = ml_dtypes.bfloat16
F32 = mybir.dt.float32
BF = mybir.dt.bfloat16
I16 = mybir.dt.int16
EXP = mybir.ActivationFunctionType.Exp
MULT = mybir.AluOpType.mult
ADD = mybir.AluOpType.add

B, T, C = 1, 4096, 1024
H, D = 16, 64
NCORES = 8
P = 128
CB = C // P          # 8 contraction blocks
KB = T // P          # 32 key blocks
QTS = T // 512       # 8 q tiles

A16C = 128.0 / np.log(2.0)
B16 = 16248.65


def is_d16(kb):
    return kb % 3 == 2 or kb >= 30   # 12 of 32 key blocks on the DVE


_cached = None


def build_bass():
    global _cached
    if _cached is not None:
        return _cached

    nc = bacc.Bacc("TRN2", target_bir_lowering=False, name="mha_v4")

    xT = nc.dram_tensor("xT", (C, T), BF, kind="ExternalInput")
    wq = nc.dram_tensor("wq", (C, P), BF, kind="ExternalInput")
    wk = nc.dram_tensor("wk", (C, P), BF, kind="ExternalInput")
    wv = nc.dram_tensor("wv", (C, P), BF, kind="ExternalInput")
    wo = nc.dram_tensor("wo", (P, C), BF, kind="ExternalInput")
    outT = nc.dram_tensor("outT", (C, T), BF, kind="ExternalOutput")

    with TileContext(nc) as tc:
        with (
            tc.tile_pool(name="const", bufs=1) as const,
            tc.tile_pool(name="work", bufs=3) as work,
            tc.tile_pool(name="psS", bufs=2, space="PSUM") as psS,
            tc.tile_pool(name="psY", bufs=1, space="PSUM") as psY,
            tc.tile_pool(name="psO", bufs=2, space="PSUM") as psO,
        ):
            # ---- load inputs (weights on gpsimd queue, x on sync queue) ----
            wqs = const.tile([P, CB, P], BF)
            nc.gpsimd.dma_start(wqs[:], wq[:, :].rearrange("(cb p) f -> p cb f", p=P))
            wks = const.tile([P, CB, P], BF)
            nc.gpsimd.dma_start(wks[:], wk[:, :].rearrange("(cb p) f -> p cb f", p=P))
            wvs = const.tile([P, CB, P], BF)
            nc.gpsimd.dma_start(wvs[:], wv[:, :].rearrange("(cb p) f -> p cb f", p=P))
            wos = const.tile([P, C], BF)
            nc.gpsimd.dma_start(wos[:], wo[:, :])
            xTs = const.tile([P, CB, T], BF)
            xT_r = xT[:, :].rearrange("(cb p) t -> p cb t", p=P)
            for tt in range(QTS):
                nc.sync.dma_start(xTs[:, :, tt * 512:(tt + 1) * 512],
                                  xT_r[:, :, tt * 512:(tt + 1) * 512])

            # ---- warm up the PE clock (HAM) during the input DMA wait ----
            warm = const.tile([P, 512], BF)
            nc.vector.memset(warm[:], 0.0)
            for _ in range(30):
                pw = psO.tile([P, 512], F32, tag="po", name="pw")
                nc.tensor.matmul(pw[:, :], warm[:, 0:P], warm[:, :],
                                 start=True, stop=True)

            qTs = const.tile([P, T], BF)
            kTs = const.tile([P, T], BF)
            eS_store = {}

            def emit_s_exp(qt, kb):
                q0 = qt * 512
                k0 = kb * P
                s = psS.tile([P, 1024], F32, tag="s", name="s")
                nc.tensor.matmul(s[:, 0:512], kTs[0:64, k0:k0 + P],
                                 qTs[0:64, q0:q0 + 512], start=True, stop=True)
                nc.tensor.matmul(s[:, 512:1024], kTs[64:128, k0:k0 + P],
                                 qTs[64:128, q0:q0 + 512], start=True, stop=True)
                eS = work.tile([P, 1024], BF, tag="es", bufs=17)
                if is_d16(kb):
                    nc.vector.tensor_scalar(eS[:].bitcast(I16), s[:], A16C, B16,
                                            op0=MULT, op1=ADD)
                else:
                    nc.scalar.activation(eS[:], s[:], EXP)
                eS_store[(qt, kb)] = eS

            LAG = 14

            Vp0 = const.tile([P, KB, 65], BF)
            Vp1 = const.tile([P, KB, 65], BF)
            nc.vector.memset(Vp0[:, :, 64:65], 1.0)
            nc.vector.memset(Vp1[:, :, 64:65], 1.0)

            # ---- projection phase: kT, qT, V per tok tile, chasing the DMA ----
            for tt in range(QTS):
                pq = psS.tile([P, 1024], F32, tag="s")
                for cb in range(CB):
                    nc.tensor.matmul(
                        pq[:, 0:512], wks[:, cb, :], xTs[:, cb, tt * 512:(tt + 1) * 512],
                        start=(cb == 0), stop=(cb == CB - 1))
                for cb in range(CB):
                    nc.tensor.matmul(
                        pq[:, 512:1024], wqs[:, cb, :], xTs[:, cb, tt * 512:(tt + 1) * 512],
                        start=(cb == 0), stop=(cb == CB - 1))
                nc.scalar.copy(kTs[:, tt * 512:(tt + 1) * 512], pq[:, 0:512])
                nc.vector.tensor_copy(qTs[:, tt * 512:(tt + 1) * 512], pq[:, 512:1024])
                for tb in range(tt * 4, tt * 4 + 4):
                    pv = psO.tile([P, 512], F32, tag="po", name="pv")
                    for cb in range(CB):
                        nc.tensor.matmul(
                            pv[:, 0:P], xTs[:, cb, tb * P:(tb + 1) * P], wvs[:, cb, :],
                            start=(cb == 0), stop=(cb == CB - 1))
                    nc.scalar.copy(Vp0[:, tb, 0:64], pv[:, 0:64])
                    nc.vector.tensor_copy(Vp1[:, tb, 0:64], pv[:, 64:128])
                # pre-emit S^T+exp pairs of q-tile 0 so ACT starts early
                if tt < 7:
                    for kb_pre in range(tt * 2, tt * 2 + 2):
                        emit_s_exp(0, kb_pre)

            # ---- fused out-projection: both heads in one K=128 matmul ----
            def emit_outproj(dep, fb, flush=False):
                yns, q0 = dep
                po = psO.tile([P, 512], F32, tag="po", name="po")
                nc.tensor.matmul(po[:, :], wos[:, fb * P:(fb + 1) * P], yns[:, :],
                                 start=True, stop=True)
                oc = work.tile([P, 512], BF, tag="oc", bufs=8)
                if flush and fb % 2 == 1:
                    nc.scalar.copy(oc[:], po[:])
                else:
                    nc.vector.tensor_copy(oc[:], po[:])
                if fb % 2 == 0:
                    nc.gpsimd.dma_start(outT[fb * P:(fb + 1) * P, q0:q0 + 512], oc[:])
                else:
                    nc.sync.dma_start(outT[fb * P:(fb + 1) * P, q0:q0 + 512], oc[:])

            OP_SLOTS = {12: 0, 14: 1, 16: 2, 18: 3, 20: 4, 22: 5, 24: 6, 26: 7}

            pending = None
            for qt in range(QTS):
                q0 = qt * 512
                y0 = psY.tile([65, 512], F32, tag="y0")
                y1 = psY.tile([65, 512], F32, tag="y1")
                for kb in range(KB):
                    tgt = qt * KB + kb + LAG
                    if tgt < QTS * KB:
                        emit_s_exp(tgt // KB, tgt % KB)
                    eS = eS_store.pop((qt, kb))
                    nc.tensor.matmul(y0[:, :], Vp0[:, kb, :], eS[:, 0:512],
                                     start=(kb == 0), stop=(kb == KB - 1))
                    nc.tensor.matmul(y1[:, :], Vp1[:, kb, :], eS[:, 512:1024],
                                     start=(kb == 0), stop=(kb == KB - 1))
                    if pending is not None and kb in OP_SLOTS:
                        emit_outproj(pending, OP_SLOTS[kb])

                # ---- free y psum fast: unnormalized casts on ACT ----
                yu0 = work.tile([64, 512], BF, tag="yu0", bufs=2)
                nc.scalar.copy(yu0[:, :], y0[0:64, :])
                yu1 = work.tile([64, 512], BF, tag="yu1", bufs=2)
                nc.scalar.copy(yu1[:, :], y1[0:64, :])
                zf = work.tile([65, 1024], F32, tag="zf", bufs=2)
                nc.vector.tensor_copy(zf[64:65, 0:512], y0[64:65, :])
                nc.vector.tensor_copy(zf[64:65, 512:1024], y1[64:65, :])

                # ---- Z -> [64,16] spread via DMA so reciprocal is parallel ----
                zsp = work.tile([64, 16], F32, tag="zsp", bufs=2)
                nc.sync.dma_start(
                    zsp[:, 0:8],
                    zf[64:65, 0:512].rearrange("o (p f) -> o p f", p=64))
                nc.sync.dma_start(
                    zsp[:, 8:16],
                    zf[64:65, 512:1024].rearrange("o (p f) -> o p f", p=64))
                zrc = work.tile([64, 16], F32, tag="zrc", bufs=2)
                nc.vector.reciprocal(zrc[:], zsp[:])
                zrw = work.tile([1, 1024], F32, tag="zrw", bufs=2)
                nc.gpsimd.dma_start(
                    zrw[0:1, 0:512].rearrange("o (p f) -> o p f", p=64),
                    zrc[:, 0:8])
                nc.gpsimd.dma_start(
                    zrw[0:1, 512:1024].rearrange("o (p f) -> o p f", p=64),
                    zrc[:, 8:16])
                zb = work.tile([64, 1024], F32, tag="zb", bufs=2)
                nc.gpsimd.partition_broadcast(zb[:, 0:512], zrw[0:1, 0:512])
                nc.gpsimd.partition_broadcast(zb[:, 512:1024], zrw[0:1, 512:1024])
                yns = work.tile([P, 512], BF, tag="yns", bufs=2)
                nc.vector.tensor_tensor(yns[0:64, :], yu0[:, :], zb[:, 0:512],
                                        op=MULT)
                y1t = work.tile([64, 512], BF, tag="y1t", bufs=2)
                nc.vector.tensor_tensor(y1t[:], yu1[:, :], zb[:, 512:1024],
                                        op=MULT)
                nc.gpsimd.dma_start(yns[64:128, :], y1t[:])
                pending = (yns, q0)

            for fb in range(CB):
                emit_outproj(pending, fb, flush=True)

    nc.compile()
    _cached = nc
    return nc


def make_in_maps(x, w_qkv, w_out):
    """x [1,T,C] f32, w_qkv [C, 3C] f32, w_out [C, C] f32 -> per-core input dicts."""
    x = np.asarray(x, dtype=np.float32)
    w_qkv = np.asarray(w_qkv, dtype=np.float32)
    w_out = np.asarray(w_out, dtype=np.float32)
    scale = 1.0 / np.sqrt(np.float32(D))
    xT = np.ascontiguousarray(x.reshape(T, C).T).astype(BF16)  # [C, T]
    in_maps = []
    for c in range(NCORES):
        cols = slice(P * c, P * (c + 1))
        wqc = np.ascontiguousarray(w_qkv[:, 0:C][:, cols] * scale).astype(BF16)
        wkc = np.ascontiguousarray(w_qkv[:, C:2 * C][:, cols]).astype(BF16)
        wvc = np.ascontiguousarray(w_qkv[:, 2 * C:3 * C][:, cols]).astype(BF16)
        woc = np.ascontiguousarray(w_out[P * c:P * (c + 1), :]).astype(BF16)
        in_maps.append({"xT": xT, "wq": wqc, "wk": wkc, "wv": wvc, "wo": woc})
    return in_maps


def run(x, w_qkv, w_out, trace=False):
    nc = build_bass()
    in_maps = make_in_maps(x, w_qkv, w_out)
    res = run_bass_kernel_spmd(nc, in_maps, core_ids=list(range(NCORES)), trace=trace)
    acc = np.zeros((C, T), dtype=np.float32)
    for r in res.results:
        acc += r["outT"].astype(np.float32)
    out = np.ascontiguousarray(acc.T).reshape(B, T, C)
    return out, res


def kernel(x, w_qkv, w_out):
    out, _ = run(x, w_qkv, w_out, trace=False)
    return out
